# revision 1
# baseline (speedup 1.0000x reference)
"""Block-circulant process via frequency-domain factorization on 8 cores.

out = x @ M factorizes through the (truncated, 48-bin) real FFT:
  stage A: per in-block j:  S[(p,f), b] = sum_t F[t,(p,f)] xT[jB+t, b]
  stage M: per freq pair e: mid[(q,i), b] = sum_{p,j} W_e[(p,j),(q,i)] S
  stage C: per out-block i: out[t, b] = sum_{q,f} G[(q,f), t] mid

All stages are single K<=128 matmuls (no PSUM accumulation). The two
partition-regroups between stages bounce through DRAM with affine
scatter APs. Sharding: pure data-parallel over batch (x dim 0), all
weight operands replicated. fp32r throughout.

PE per core: 88 matmuls (~20us). HBM per core: ~41 MiB.
"""

import numpy as np

B = 128
K_HALF = B // 2 + 1  # 65
KT = 48  # frequency truncation
KI = 32
KO = 32
BATCH = 4096
IN_F = 4096
OUT_F = 4096

N_CORES = 8
BQ = BATCH // N_CORES  # 512 batch rows per core
NP = KT // 2  # 24 frequency pairs
FE = NP  # e index range

_CACHE = {}
LAST_RESULTS = None
TRACE = False


def _build_nc():
    import concourse.bacc as bacc
    import concourse.mybir as mybir
    import concourse.tile as tile

    F32R = mybir.dt.float32r
    F32 = mybir.dt.float32

    nc = bacc.Bacc(None, target_bir_lowering=False)
    xT = nc.declare_dram_parameter("xT", [IN_F, BQ], F32R, isOutput=False)
    fmat = nc.declare_dram_parameter("fmat", [128, 96], F32R, isOutput=False)
    gmat = nc.declare_dram_parameter("gmat", [96, 128], F32R, isOutput=False)
    wmid = nc.declare_dram_parameter("wmid", [128, NP * 128], F32R,
                                     isOutput=False)
    oT = nc.declare_dram_parameter("oT", [OUT_F, BQ], F32, isOutput=True)

    # DRAM intermediates, laid out so stages M and C each load their whole
    # input with ONE contiguous DMA (48/64KB partition lines)
    # sS[fl*64 + p*32 + j, e*BQ + b]
    sS = nc.dram_tensor("sS", [128, NP * BQ], F32R)
    # cmid[q*48 + f, i*BQ + b]
    cmid = nc.dram_tensor("cmid", [96, KO * BQ], F32R)

    # views for the scattered writes
    sS_v = sS.rearrange("(fl p j) (e b) -> fl j p e b", fl=2, p=2, e=NP)
    cmid_v = cmid.rearrange("(q fe fl) (i b) -> fl fe q i b", fl=2, fe=FE,
                            i=KO)

    with tile.TileContext(nc) as tc:
        with (
            tc.tile_pool(name="cpool", bufs=1) as cpool,
            tc.tile_pool(name="xpool", bufs=8) as xpool,
            tc.tile_pool(name="spool", bufs=24) as spool,
            tc.tile_pool(name="bigpool", bufs=3) as bigpool,
            tc.tile_pool(name="opool", bufs=10) as opool,
            tc.tile_pool(name="psum", bufs=3, space="PSUM") as psum,
            tc.tile_pool(name="psum2", bufs=2, space="PSUM") as psum2,
        ):
            f_t = cpool.tile([128, 96], F32R, name="f_t")
            nc.sync.dma_start(f_t[:], fmat[:])
            g_t = cpool.tile([96, 128], F32R, name="g_t")
            nc.sync.dma_start(g_t[:], gmat[:])
            # all 24 middle weight blocks in one DMA
            w_all = cpool.tile([128, NP * 128], F32R, name="w_all")
            nc.sync.dma_start(w_all[:], wmid[:])

            lanes = [nc.scalar, nc.sync, nc.gpsimd]

            # ---- stage A: 32 matmuls + scattered DRAM writes ----
            for j in range(KI):
                x_t = xpool.tile([128, BQ], F32R, name="x_t")
                (nc.sync if j % 2 == 0 else nc.scalar).dma_start(
                    x_t[:], xT[j * 128:(j + 1) * 128, :])
                ps = psum.tile([96, BQ], mybir.dt.float32, name="ps_a",
                               tag="ps_a")
                nc.tensor.matmul(ps[:], f_t[:], x_t[:], start=True, stop=True)
                s_t = spool.tile([96, BQ], F32, name="s_t")
                nc.vector.tensor_copy(s_t[:], ps[:])
                for fl in range(2):
                    nc.gpsimd.dma_start(
                        sS_v[fl, j],
                        s_t[fl * 48:(fl + 1) * 48, :].bitcast(F32R),
                    )

            # ---- stage M: grouped reads (6 pairs/DMA) + 24 matmuls ----
            EG = 6
            for g in range(NP // EG):
                m_g = bigpool.tile([128, EG * BQ], F32R, name="m_g",
                                   tag="big")
                nc.sync.dma_start(m_g[:], sS[:, g * EG * BQ:(g + 1) * EG * BQ])
                for ee in range(EG):
                    e = g * EG + ee
                    ps = psum2.tile([128, BQ], mybir.dt.float32,
                                    name="ps_m", tag="ps_m")
                    nc.tensor.matmul(ps[:], w_all[:, e * 128:(e + 1) * 128],
                                     m_g[:, ee * BQ:(ee + 1) * BQ],
                                     start=True, stop=True)
                    m_out = opool.tile([128, BQ], F32, name="m_out", tag="mo")
                    nc.vector.tensor_copy(m_out[:], ps[:])
                    for fl in range(2):
                        (nc.scalar if fl == 0 else nc.gpsimd).dma_start(
                            cmid_v[fl, e],
                            m_out[fl * 64:(fl + 1) * 64, :].bitcast(F32R),
                        )

            # ---- stage C: grouped reads (8 i/DMA) + 32 matmuls ----
            IG = 8
            for g in range(KO // IG):
                c_g = bigpool.tile([96, IG * BQ], F32R, name="c_g", tag="big")
                nc.sync.dma_start(c_g[:],
                                  cmid[:, g * IG * BQ:(g + 1) * IG * BQ])
                for ii in range(IG):
                    i = g * IG + ii
                    ps = psum.tile([128, BQ], mybir.dt.float32, name="ps_c",
                                   tag="ps_c")
                    nc.tensor.matmul(ps[:], g_t[:],
                                     c_g[:, ii * BQ:(ii + 1) * BQ],
                                     start=True, stop=True)
                    o_t = opool.tile([128, BQ], F32, name="o_t")
                    nc.vector.tensor_copy(o_t[:], ps[:])
                    (nc.scalar if i % 2 == 0 else nc.gpsimd).dma_start(
                        oT[i * 128:(i + 1) * 128, :], o_t[:])
    nc.finalize()
    return nc


def _get_nc():
    if "nc" not in _CACHE:
        _CACHE["nc"] = _build_nc()
    return _CACHE["nc"]


def _host_weights(W_real, W_imag):
    """F [128,96], G [96,128], Wmid [24,128,128] (all float32)."""
    t = np.arange(B)[:, None].astype(np.float64)
    # F columns ordered (fl, p, e): f = 2e + fl; p=0 -> cos, p=1 -> -sin
    F = np.zeros((128, 96))
    for fl in range(2):
        for p in range(2):
            for e in range(FE):
                f = 2 * e + fl
                col = fl * 48 + p * 24 + e
                w = 2 * np.pi * f * t[:, 0] / B
                F[:, col] = np.cos(w) if p == 0 else -np.sin(w)
    # G rows ordered (q, f): q=0 -> scale*cos, q=1 -> -scale*sin
    G = np.zeros((96, 128))
    fs = np.arange(KT)
    scale = np.full(KT, 2.0 / B)
    scale[0] = 1.0 / B
    for q in range(2):
        for f in range(KT):
            w = 2 * np.pi * f * np.arange(B) / B
            G[q * 48 + f] = (scale[f] * np.cos(w) if q == 0
                             else -scale[f] * np.sin(w))
    # Wmid[e]: rows (fl, p, j), cols (fl, q, i); block-diag in fl
    Wr = W_real.astype(np.float64)
    Wi = W_imag.astype(np.float64)
    Wm = np.zeros((NP, 128, 128))
    for e in range(NP):
        for fl in range(2):
            f = 2 * e + fl
            r0, c0 = fl * 64, fl * 64
            # q=0: Re_out = Wr @ Re + Wi @ Im ; q=1: Im_out = Wr @ Im - Wi @ Re
            # rows (p=0: Re-in j), (p=1: Im-in j); cols (q, i)
            # lhsT[(p,j),(q,i)]: value multiplying S[p,j] into out[q,i]
            Wrf = Wr[:, :, f].T  # [j, i]
            Wif = Wi[:, :, f].T
            Wm[e, r0:r0 + 32, c0:c0 + 32] = Wrf          # p0 -> q0: Wr
            Wm[e, r0 + 32:r0 + 64, c0:c0 + 32] = Wif     # p1 -> q0: Wi
            Wm[e, r0:r0 + 32, c0 + 32:c0 + 64] = -Wif    # p0 -> q1: -Wi
            Wm[e, r0 + 32:r0 + 64, c0 + 32:c0 + 64] = Wrf  # p1 -> q1: Wr
    return (F.astype(np.float32), G.astype(np.float32), Wm.astype(np.float32))


def kernel(x, W_real, W_imag):
    global LAST_RESULTS
    from concourse.bass_utils import run_bass_kernel_spmd

    x = np.asarray(x, dtype=np.float32)
    F, G, Wm = _host_weights(np.asarray(W_real), np.asarray(W_imag))
    xt = np.ascontiguousarray(x.T)  # (IN_F, BATCH)

    in_maps = []
    for core in range(N_CORES):
        xT_shard = np.ascontiguousarray(xt[:, core * BQ:(core + 1) * BQ])
        wm_packed = np.ascontiguousarray(
            Wm.transpose(1, 0, 2).reshape(128, NP * 128))
        in_maps.append(
            {"xT": xT_shard, "fmat": F, "gmat": G, "wmid": wm_packed})

    nc = _get_nc()
    res = run_bass_kernel_spmd(nc, in_maps, list(range(N_CORES)), trace=TRACE)
    LAST_RESULTS = res

    out = np.empty((BATCH, OUT_F), np.float32)
    for core in range(N_CORES):
        out[core * BQ:(core + 1) * BQ, :] = res.results[core]["oT"].T
    return out

